# revision 54
# baseline (speedup 1.0000x reference)
"""Multi-head causal attention (B=2, S=2048, D=1024, H=16) on 8 trn2 cores.

Sharding: tensor-parallel over heads. Each core owns 2 heads: a 128-column
slice of w_q/w_k/w_v and the matching 128-row slice of w_o. Every core
computes a full [B*S, D] partial output; the host sums the 8 partials and
adds the bias.

Per-core kernel (all matmuls in float32r -> full PE rate), fully pipelined
over s-chunks of 512:
  Q-proj(chunk) -> attention non-diag ks-tiles (need only this chunk's Q +
  earlier chunks' K/V, so they overlap this chunk's own K/V projections and
  V transposes) -> K/V-proj + V-transpose(chunk) -> deferred out-proj of the
  previous chunk -> attention diagonal tail -> batched 2MB store, with Tile
  overlapping everything via subregion deps. Chunk input loads are per-k-tile
  DMAs on the SP ring; stores/weights ride the ACT ring.

  - QT/KT/VT [128, 4096] from w.T-tiles (stationary) x xT-chunks (moving).
    x is pre-transposed on the host so the contraction dim is on partitions.
  - V re-laid-out to [seq, d] via PE transpose; each head's V gets a ones
    column appended so the AV matmul's PSUM row 64 accumulates the softmax
    denominator for free (M=65, heads sequential).
  - scoresT[ks, qs] matmul pair packed into disjoint PE row-groups
    (contraction is 64); both land in one 2-bank PSUM tile so a single ACT
    exp (scale=1/8 folds the 1/sqrt(hd)) covers both heads, halving ACT's
    per-instruction overhead. No max-subtraction (scores provably small).
  - causal masking: gpsimd affine_select zeroes the invalid half of the 4
    diagonal-chunk ET tiles (exp first, zero after -- exact).
  - ctx eviction divides by the denominator row (DVE mult by gpsimd-broadcast
    reciprocal); head 1 is DMA-shifted to partitions 64..127.
  - out-proj ctxT.T @ w_o_c per chunk, PSUM rotated through the idle proj
    slots, evicted by DVE, streamed to DRAM while attention continues.
"""

import sys

sys.path.insert(0, "/opt/trn_rl_repo")

import numpy as np

import concourse.bass as bass
import concourse.mybir as mybir
import concourse.tile as tile
from concourse import bacc
from concourse.bass_utils import run_bass_kernel_spmd

B, S, D, H, HD = 2, 2048, 1024, 16, 64
BS = B * S                  # 4096 flattened rows
NCORES = 8
DC = D // NCORES            # 128 head-dims per core (2 heads)
P = 128                     # partitions
SC = 512                    # s-chunk (moving free dim)
NSC = BS // SC              # 8 s-chunks over the flattened rows
NKT = D // P                # 8 k-tiles for the projections
NQC = S // SC               # 4 q-chunks per batch
NST = BS // P               # 32 s-tiles of 128
SPB = S // P                # 16 s-tiles per batch

F32 = mybir.dt.float32
F32R = mybir.dt.float32r

LABELS = {}


def _lbl(bi, label):
    try:
        LABELS[bi.ins.name] = label
    except Exception:
        pass
    return bi


def _build_nc(phases=("proj", "attn", "oproj")):
    nc = bacc.Bacc(None, target_bir_lowering=False)

    xT = nc.dram_tensor("xT", [D, BS], F32R, kind="ExternalInput")
    wq = nc.dram_tensor("wq", [D, DC], F32R, kind="ExternalInput")
    wk = nc.dram_tensor("wk", [D, DC], F32R, kind="ExternalInput")
    wv = nc.dram_tensor("wv", [D, DC], F32R, kind="ExternalInput")
    wo = nc.dram_tensor("wo", [DC, D], F32R, kind="ExternalInput")
    ident_d = nc.dram_tensor("ident", [P, P], F32R, kind="ExternalInput")
    out = nc.dram_tensor("out", [BS, D], F32, kind="ExternalOutput")

    with tile.TileContext(nc) as tc:
        with (
            tc.tile_pool(name="big", bufs=1) as big,
            tc.tile_pool(name="xts", bufs=2) as xts,
            tc.tile_pool(name="ob", bufs=2) as obs,
            tc.tile_pool(name="et", bufs=5) as etp,
            tc.tile_pool(name="small", bufs=2) as small,
            tc.tile_pool(name="ps_a", bufs=2, space="PSUM") as ps_a,   # proj + oproj [128,512]
            tc.tile_pool(name="ps_b", bufs=2, space="PSUM") as ps_b,   # score pairs [128,2,512] + vtr
            tc.tile_pool(name="ps_c", bufs=1, space="PSUM") as ps_c,   # ctx pair [65,2,512]
        ):
            qt = big.tile([P, BS], F32R, tag="qt")
            kt = big.tile([P, BS], F32R, tag="kt")
            vt = big.tile([P, BS], F32R, tag="vt")
            ctxT = big.tile([P, BS], F32R, tag="ctxT")
            vone = big.tile([P, 2, NST, 65], F32R, tag="vone")
            wq_sb = big.tile([P, NKT, DC], F32R, tag="wq")
            wk_sb = big.tile([P, NKT, DC], F32R, tag="wk")
            wv_sb = big.tile([P, NKT, DC], F32R, tag="wv")
            wo_sb = big.tile([P, D], F32R, tag="wo")
            ident = big.tile([P, P], F32R, tag="ident")

            nc.scalar.dma_start(wq_sb[:], wq.rearrange("(t p) m -> p t m", p=P))
            nc.scalar.dma_start(wk_sb[:], wk.rearrange("(t p) m -> p t m", p=P))
            nc.scalar.dma_start(wv_sb[:], wv.rearrange("(t p) m -> p t m", p=P))
            nc.scalar.dma_start(wo_sb[:], wo[:])
            nc.scalar.dma_start(ident[:], ident_d[:])
            nc.gpsimd.memset(vone[:].bitcast(F32), 1.0)

            xT_r = xT.rearrange("(t p) s -> t p s", p=P)

            def do_proj_q(sc):
                """Input chunk DMA + Q projection for s-chunk sc."""
                cols = slice(sc * SC, (sc + 1) * SC)
                xt_t = xts.tile([P, NKT, SC], F32R, tag="xt")
                for kq in range(NKT):
                    nc.sync.dma_start(xt_t[:, kq:kq+1, :], xT_r[kq:kq+1, :, cols].transpose([1, 0, 2]))
                psp = ps_a.tile([P, SC], F32, tag="proj")
                for k in range(NKT):
                    _lbl(nc.tensor.matmul(psp[:], wq_sb[:, k, :], xt_t[:, k, :],
                                          start=(k == 0), stop=(k == NKT - 1)),
                         f"proj{sc}")
                nc.scalar.copy(qt[:, cols], psp[:])
                return xt_t

            def do_proj_kv(sc, xt_t):
                """K/V projections + V transpose for s-chunk sc."""
                cols = slice(sc * SC, (sc + 1) * SC)
                for w_sb, dst in ((wk_sb, kt), (wv_sb, vt)):
                    psp = ps_a.tile([P, SC], F32, tag="proj")
                    for k in range(NKT):
                        _lbl(nc.tensor.matmul(psp[:], w_sb[:, k, :], xt_t[:, k, :],
                                              start=(k == 0), stop=(k == NKT - 1)),
                             f"proj{sc}")
                    nc.scalar.copy(dst[:, cols], psp[:])
                # V transpose for the 4 s-tiles of this chunk
                for gg in range(4):
                    g = sc * 4 + gg
                    psT = ps_b.tile([P, 2, SC], F32R, tag="sc")
                    _lbl(nc.tensor.transpose(psT[:, 0, 0:P], vt[:, g * P:(g + 1) * P], ident[:]), f"vtr{sc}")
                    nc.scalar.copy(vone[:, 0, g, 0:64], psT[:, 0, 0:64])
                    nc.scalar.copy(vone[:, 1, g, 0:64], psT[:, 0, 64:128])

            def do_ks_tile(b, j, t, psc):
                nks = 4 * (j + 1)
                g = b * SPB + t
                kcols = slice(g * P, (g + 1) * P)
                diag = t >= nks - 4
                mi = t - (nks - 4) if diag else 0
                v0 = mi * P            # first possibly-valid qs column
                w0 = min(v0, 256)      # matmul restriction (keep N >= 256)
                qw = slice(b * S + j * SC + w0, b * S + (j + 1) * SC)
                ps_s = ps_b.tile([P, 2, SC], F32, tag="sc")
                for h in range(2):
                    hp = slice(h * 64, (h + 1) * 64)
                    _lbl(nc.tensor.matmul(
                        ps_s[:, h, w0:], kt[hp, kcols], qt[hp, qw],
                        start=True, stop=True, tile_position=(h * 64, 0),
                    ), f"score b{b}j{j}t{t}")
                et = etp.tile([P, 2, SC], F32R, tag="et")
                if v0 > 0:
                    nc.gpsimd.memset(et[:, :, 0:v0].bitcast(F32), 0.0)
                nc.scalar.activation(
                    et[:, :, v0:], ps_s[:, :, v0:],
                    mybir.ActivationFunctionType.Exp, scale=0.125,
                )
                if diag:
                    nc.gpsimd.affine_select(
                        out=et[:, :, v0:], in_=et[:, :, v0:],
                        compare_op=mybir.AluOpType.is_ge,
                        fill=0.0, base=0,
                        pattern=[[0, 2], [1, SC - v0]], channel_multiplier=-1,
                    )
                for h in range(2):
                    _lbl(nc.tensor.matmul(
                        psc[:, h, w0:], vone[:, h, g, :], et[:, h, w0:],
                        start=(t == 0), stop=(t == nks - 1),
                    ), f"av b{b}j{j}t{t}")

            def do_attn_head(b, j):
                """Non-diagonal ks-tiles of q-chunk (b, j): need only chunk j's Q
                plus previous chunks' K/V -- runs while chunk j's K/V project."""
                psc = ps_c.tile([65, 2, SC], F32, tag="ctx")
                for t in range(4 * (j + 1) - 4):
                    do_ks_tile(b, j, t, psc)
                return psc

            def do_attn_tail(b, j, psc):
                """Diagonal ks-tiles + eviction/normalization for q-chunk (b, j)."""
                qcols = slice(b * S + j * SC, b * S + (j + 1) * SC)
                nks = 4 * (j + 1)
                for t in range(nks - 4, nks):
                    do_ks_tile(b, j, t, psc)
                # fast raw eviction frees the PSUM slot; normalize afterwards
                tmp = small.tile([65, 2, SC], F32, tag="tmp")
                nc.scalar.copy(tmp[:], psc[:])
                for h in range(2):
                    rec = small.tile([1, SC], F32, tag="rec")
                    recb = small.tile([64, SC], F32, tag="recb")
                    nc.vector.reciprocal(rec[:], tmp[64:65, h, :])
                    nc.gpsimd.partition_broadcast(recb[:], rec[:])
                    if h == 0:
                        nc.vector.tensor_mul(ctxT[0:64, qcols], tmp[0:64, h, :], recb[:])
                    else:
                        stg = small.tile([64, SC], F32R, tag="stg")
                        nc.vector.tensor_mul(stg[:], tmp[0:64, h, :], recb[:])
                        nc.scalar.dma_start(ctxT[64:128, qcols], stg[:])

            def do_oproj_chunk(b, j):
                """Out-proj + batched 2MB store (ACT ring) for q-chunk (b, j)."""
                st0 = (b * S + j * SC) // P
                ob = obs.tile([P, 4, 2, SC], F32, tag="ob")
                out_view = out.rearrange("(g p) (j f) -> p g j f", p=P, j=2)
                for st4 in range(4):
                    st = st0 + st4
                    for jo in range(2):
                        pso = ps_a.tile([P, SC], F32, tag="proj")
                        _lbl(nc.tensor.matmul(
                            pso[:], ctxT[:, st * P:(st + 1) * P],
                            wo_sb[:, jo * SC:(jo + 1) * SC],
                            start=True, stop=True,
                        ), f"oproj b{b}j{j}st{st4}jo{jo}")
                        nc.vector.tensor_copy(ob[:, st4, jo, :], pso[:])
                nc.scalar.dma_start(out_view[:, st0:st0 + 4, :, :], ob[:])

            # pipeline over s-chunks. proj chunk sc unlocks attention chunk
            # (b, j) with b*NQC+j == sc. The chunk's non-diagonal ks-tiles only
            # need chunk sc's Q (emitted first) + previous chunks' K/V, so they
            # overlap chunk sc's own K/V projections and V transposes; the
            # diagonal tail follows. Out-proj of the previous attention chunk
            # is deferred past the next proj chunk so its PSUM-slot reuse never
            # stalls the critical path.
            pending_oproj = []
            for sc in range(NSC if "proj" in phases else 0):
                b, j = sc // NQC, sc % NQC
                xt_t = do_proj_q(sc)
                psc = do_attn_head(b, j) if "attn" in phases else None
                do_proj_kv(sc, xt_t)
                if pending_oproj and "oproj" in phases:
                    do_oproj_chunk(*pending_oproj.pop(0))
                if "attn" in phases:
                    do_attn_tail(b, j, psc)
                    pending_oproj.append((b, j))
            if "oproj" in phases:
                for bj in pending_oproj:
                    do_oproj_chunk(*bj)

    nc.compile()
    return nc


_NC_CACHE = None


def _get_nc():
    global _NC_CACHE
    if _NC_CACHE is None:
        _NC_CACHE = _build_nc()
    return _NC_CACHE


def kernel(x, w_q, w_k, w_v, w_o, b_o):
    x = np.asarray(x, dtype=np.float32)
    w_q = np.asarray(w_q, dtype=np.float32)
    w_k = np.asarray(w_k, dtype=np.float32)
    w_v = np.asarray(w_v, dtype=np.float32)
    w_o = np.asarray(w_o, dtype=np.float32)
    b_o = np.asarray(b_o, dtype=np.float32)

    xT = np.ascontiguousarray(x.reshape(BS, D).T)

    nc = _get_nc()
    in_maps = []
    for c in range(NCORES):
        cols = slice(c * DC, (c + 1) * DC)
        in_maps.append({
            "xT": xT,
            "ident": np.eye(P, dtype=np.float32),
            "wq": np.ascontiguousarray(w_q[:, cols]),
            "wk": np.ascontiguousarray(w_k[:, cols]),
            "wv": np.ascontiguousarray(w_v[:, cols]),
            "wo": np.ascontiguousarray(w_o[cols, :]),
        })

    res = None
    for attempt in range(3):
        try:
            res = run_bass_kernel_spmd(nc, in_maps, list(range(NCORES)))
            break
        except Exception:
            if attempt == 2:
                raise
            import time
            time.sleep(2.0)
    acc = res.results[0]["out"].astype(np.float32)
    for c in range(1, NCORES):
        acc = acc + res.results[c]["out"]
    acc = acc + b_o[None, :]
    return acc.reshape(B, S, D)
